# revision 58
# baseline (speedup 1.0000x reference)
"""GCATopo (2-layer GTAT GNN) Trainium2 kernel, 8-way SPMD.

Strategy:
 - Nodes partitioned into 8 contiguous ranges (one per core). Edges are
   assigned to the core that owns their dst node, sorted by dst, padded so
   every 128-dst-node block starts at a fresh 128-edge tile. Per-block tile
   counts are shared across cores (SPMD: one program, per-core data).
 - Per layer, each core computes for its node slice a packed "ext" row
   [xl(512) | topo(15) | 1.0 | al(4) | ta(4) | pad] = 576 f32-equiv (1280B
   bf16 row with an f32 payload block) via dense bf16 matmuls (attention
   logit weights folded into the same matmuls), then an AllGather
   replicates the ext table to every core (the halo exchange).
 - Edge phase (St-stationary form): dma_gather pulls src-node ext rows per
   128-edge tile; logits/exp are computed batched across a whole gather
   group (up to 8 tiles per DVE/Act instruction); the weighted message
   matrix R = [feat*e2 | topo*e1 | e2] is built with broadcast multiplies;
   the segment-sum runs on the tensor engine with the one-hot dst-selection
   matrix St as the STATIONARY operand (2 matmuls per tile), producing
   node-major [dst, chan] partials in PSUM; softmax denominators accumulate
   in the same matmuls; normalization becomes per-partition scalar math.
 - Final: mean-pool partials + AllReduce + tiny MLP heads on every core.
"""

from contextlib import ExitStack

import numpy as np

import concourse.bass as bass
import concourse.bacc as bacc
import concourse.tile as tile
from concourse import mybir
from concourse.masks import make_identity
from concourse.bass_utils import run_bass_kernel_spmd

F32 = mybir.dt.float32
BF16 = mybir.dt.bfloat16
I16 = mybir.dt.int16
AF = mybir.ActivationFunctionType
OP = mybir.AluOpType

P = 128


class Cfg:
    def __init__(self, N=20000, E=240000, FIN=576, HID=128, TOPO=15, H=4,
                 CORES=8, NEG=0.2):
        self.N, self.E, self.FIN, self.TOPO, self.H = N, E, FIN, TOPO, H
        self.HID = HID
        self.CORES, self.NEG = CORES, NEG
        self.HC = H * HID                      # 512
        self.ROW = self.HC + 128               # packed ext row (bf16), 1280B
        assert (self.ROW * 2) % 256 == 0
        self.NPC = N // CORES                  # nodes per core
        self.NBLK = (self.NPC + P - 1) // P    # dst blocks per core
        # f32 payload lives in bf16 slots [HC : HC+64) -> f32 view [0:32)
        self.C_TOPO = 0                        # in the f32 view
        self.C_ONE = TOPO
        self.C_AL = 16
        self.C_TA = 20
        self.DATT = 64                         # dst_att row (f32), 256B
        self.SMW = 4 * 16 + H                  # 68: [topo*e1|s1]x4 + s2x4


CFG = Cfg()
GT_MAX = 8  # max tiles (=128 idxs each) per gather call; HW rejects >~1k idxs


def cdiv(a, b):
    return (a + b - 1) // b


def ktiles(F):
    return [(o, min(P, F - o)) for o in range(0, F, P)]


# --------------------------------------------------------------------------
# host-side graph preprocessing (pure indexing on edge_index)
# --------------------------------------------------------------------------

def host_prep(edge_index, cfg):
    N, CORES, NPC, NBLK = cfg.N, cfg.CORES, cfg.NPC, cfg.NBLK
    src = np.asarray(edge_index[0], dtype=np.int64)
    dst = np.asarray(edge_index[1], dtype=np.int64)
    loops = np.arange(N, dtype=np.int64)
    src = np.concatenate([src, loops])
    dst = np.concatenate([dst, loops])
    order = np.argsort(dst, kind="stable")
    s, d = src[order], dst[order]

    core_of = d // NPC
    blk_of = (d % NPC) // P
    counts = np.zeros((CORES, NBLK), dtype=np.int64)
    for c in range(CORES):
        m = core_of == c
        bb = blk_of[m]
        for b in range(NBLK):
            counts[c, b] = int((bb == b).sum())
    schedule = [max(1, cdiv(int(counts[:, b].max()), P)) for b in range(NBLK)]
    offs = np.concatenate([[0], np.cumsum(schedule)]).astype(np.int64)
    ttot = int(offs[-1])

    srcidx = np.zeros((CORES, ttot * P), dtype=np.int16)
    dstidx = np.zeros((CORES, ttot * P), dtype=np.int16)
    dstloc = np.full((CORES, ttot * P), -1.0, dtype=np.float32)
    for c in range(CORES):
        m = core_of == c
        sc, dc, bc = s[m], d[m], blk_of[m]
        for b in range(NBLK):
            mb = bc == b
            n = int(mb.sum())
            base = int(offs[b]) * P
            srcidx[c, base:base + n] = sc[mb].astype(np.int16)
            dstidx[c, base:base + n] = (dc[mb] - c * NPC).astype(np.int16)
            dstloc[c, base:base + n] = (dc[mb] - (c * NPC + b * P)).astype(np.float32)

    # wrap for dma_gather: index i lives at [i % 16, i // 16]; the 16-row
    # block is replicated 8x along partitions (one stripe per gpsimd core)
    src_w = [np.tile(srcidx[c].reshape(-1, 16).T, (8, 1)).copy()
             for c in range(CORES)]
    dst_w = [np.tile(dstidx[c].reshape(-1, 16).T, (8, 1)).copy()
             for c in range(CORES)]
    # dstloc: edge j -> tile j//128, partition j%128
    dl_m = [dstloc[c].reshape(ttot, P).T.copy() for c in range(CORES)]
    return schedule, src_w, dst_w, dl_m


def host_attB(att, cfg):
    """att [1,H,C] -> block-diag [H*C, H] (pure placement of input values)."""
    H, C = cfg.H, cfg.HID
    out = np.zeros((H * C, H), dtype=np.float32)
    a = np.asarray(att, dtype=np.float32).reshape(H, C)
    for h in range(H):
        out[h * C:(h + 1) * C, h] = a[h]
    return out


# --------------------------------------------------------------------------
# program builder
# --------------------------------------------------------------------------

class Prog:
    pass


def build_program(cfg, schedule):
    es = ExitStack()
    nc = bacc.Bacc("TRN2", target_bir_lowering=False, debug=False,
                   num_devices=cfg.CORES, num_swdge_queues=2)
    pr = Prog()
    pr.nc = nc
    N, FIN, HID, TOPO, H, HC, ROW, NPC, NBLK = (
        cfg.N, cfg.FIN, cfg.HID, cfg.TOPO, cfg.H, cfg.HC, cfg.ROW, cfg.NPC,
        cfg.NBLK)
    SMW = cfg.SMW
    TTOT = sum(schedule)
    TMAX = max(schedule)
    W16 = TTOT * P // 16
    groups = [list(range(cfg.CORES))]

    def din(name, shape, dtype=F32):
        return nc.dram_tensor(name, list(shape), dtype, kind="ExternalInput")

    # ---- external inputs ----
    x_sl = din("x_slice", (NPC, FIN))
    te_w1 = din("te_w1", (FIN, HID)); te_b1 = din("te_b1", (HID,))
    te_w2 = din("te_w2", (HID, TOPO)); te_b2 = din("te_b2", (TOPO,))
    wts = {}
    for L in (1, 2):
        KIN = FIN if L == 1 else HC
        wts[L] = dict(
            wl=din(f"l{L}_wl", (KIN, HC)), bl=din(f"l{L}_bl", (HC,)),
            wr=din(f"l{L}_wr", (KIN, HC)), br=din(f"l{L}_br", (HC,)),
            attB=din(f"l{L}_attB", (HC, H)),
            attBp=din(f"l{L}_attBp", (HC, H)),
            att2T=din(f"l{L}_att2T", (TOPO, H)),
            bias=din(f"l{L}_bias", (HC,)), bias2=din(f"l{L}_bias2", (TOPO,)),
        )
    heads = {}
    for nm in ("v", "a"):
        heads[nm] = dict(w1=din(f"{nm}_w1", (HC, HID)), b1=din(f"{nm}_b1", (HID,)),
                         w2=din(f"{nm}_w2", (HID, 1)), b2=din(f"{nm}_b2", (1,)))
    src_i = din("src_idx", (P, W16), I16)
    dst_i = din("dst_idx", (P, W16), I16)
    dl_i = din("dstloc", (P, TTOT))

    # ---- outputs ----
    val_o = nc.dram_tensor("valence", [1, 1], F32, kind="ExternalOutput")
    aro_o = nc.dram_tensor("arousal", [1, 1], F32, kind="ExternalOutput")

    # ---- internal DRAM ----
    ext_sl = [nc.dram_tensor(f"ext_slice{L}", [NPC, ROW], BF16)
              for L in (1, 2)]
    ext_fl = [nc.dram_tensor(f"ext_full{L}", [N, ROW], BF16,
                             addr_space="Shared") for L in (1, 2)]
    datt_sl = [nc.dram_tensor(f"datt_slice{L}", [NPC, cfg.DATT], F32)
               for L in (1, 2)]
    pool_in = nc.dram_tensor("pool_in", [1, HC], F32)
    pool_out = nc.dram_tensor("pool_out", [cfg.CORES, HC], F32,
                              addr_space="Shared")
    pool_sum = nc.dram_tensor("pool_sum", [1, HC], F32)

    blocks = ktiles(NPC)          # node chunks (== dst blocks) per core
    fkt = ktiles(FIN)
    ckt = ktiles(HC)
    offs = np.concatenate([[0], np.cumsum(schedule)]).astype(int)

    ctx_noncontig = nc.allow_non_contiguous_dma("tiny transposed weight loads")
    ctx_noncontig.__enter__()
    with tile.TileContext(nc) as tc:
        # ================= static SBUF =================
        ident = nc.alloc_sbuf_tensor("ident", [P, P], F32).ap()
        make_identity(nc, ident)
        ident_bf = nc.alloc_sbuf_tensor("ident_bf", [P, P], BF16).ap()
        nc.vector.tensor_copy(ident_bf, ident)
        iota_i = nc.alloc_sbuf_tensor("iota_i", [P, P], mybir.dt.int32).ap()
        nc.gpsimd.iota(iota_i, pattern=[[1, P]], base=0, channel_multiplier=0)
        iota_bf = nc.alloc_sbuf_tensor("iota_bf", [P, P], BF16).ap()
        nc.vector.tensor_copy(iota_bf, iota_i)
        ones_col_bf = nc.alloc_sbuf_tensor("ones_col_bf", [P, 1], BF16).ap()
        nc.gpsimd.memset(ones_col_bf, 1.0)
        ones_bf = nc.alloc_sbuf_tensor("ones_bf", [1, NPC], BF16).ap()
        nc.gpsimd.memset(ones_bf, 1.0)

        src_sb = nc.alloc_sbuf_tensor("src_sb", [P, W16], I16).ap()
        dst_sb = nc.alloc_sbuf_tensor("dst_sb", [P, W16], I16).ap()
        dl_sb = nc.alloc_sbuf_tensor("dl_sb", [P, TTOT], F32).ap()
        nc.sync.dma_start(src_sb, src_i[:, :])
        nc.sync.dma_start(dst_sb, dst_i[:, :])
        nc.sync.dma_start(dl_sb, dl_i[:, :])
        dl_bf = nc.alloc_sbuf_tensor("dl_bf", [P, TTOT], BF16).ap()
        nc.vector.tensor_copy(dl_bf, dl_sb)

        topoT = [nc.alloc_sbuf_tensor(f"topoT{L}", [TOPO, NPC], F32).ap()
                 for L in (1, 2)]
        hfm = [nc.alloc_sbuf_tensor(f"hfm{h}", [P, NPC], BF16).ap()
               for h in range(H)]
        b2col = nc.alloc_sbuf_tensor("b2col", [TOPO, 1], F32).ap()
        nc.sync.dma_start(b2col, wts[1]["bias2"][:].rearrange("(t o) -> t o", o=1))
        bcol = {}
        for L in (1, 2):
            bcol[L] = nc.alloc_sbuf_tensor(f"bcol{L}", [P, H], F32).ap()
            nc.sync.dma_start(bcol[L],
                              wts[L]["bias"][:].rearrange("(h c) -> c h", h=H))

        # =========================================================
        def emit_aux_weights(L, w, KIN, wl_sb, wp, pp):
            """[wl@attB | wr@attB] k-tiles [(fk,8)] bf16 + bias row [1,8].

            wl_sb: already-loaded bf16 wl k-tiles (reused as transpose src).
            Layout: cols 0:4 = al weights, 4:8 = ar weights.
            """
            kk = ktiles(KIN)
            nk = len(kk)
            with tc.tile_pool(name=f"aux{KIN}", bufs=2) as ap_:
                pw = pp.tile([P, nk * 8 + 8], F32, tag="auxw", name="auxw", space="PSUM", bufs=1)
                blc = ap_.tile([P, HC // P], BF16, tag="blc", name="blc")
                blcf = ap_.tile([P, HC // P], F32, tag="blcf", name="blcf")
                nc.sync.dma_start(blcf[:, :],
                                  w["bl"][:].rearrange("(a c) -> c a", c=P))
                nc.vector.tensor_copy(blc[:, :], blcf[:, :])
                brc = ap_.tile([P, HC // P], BF16, tag="brc", name="brc")
                brcf = ap_.tile([P, HC // P], F32, tag="brcf", name="brcf")
                nc.sync.dma_start(brcf[:, :],
                                  w["br"][:].rearrange("(a c) -> c a", c=P))
                nc.vector.tensor_copy(brc[:, :], brcf[:, :])
                nfull = KIN // P
                wr_f = ap_.tile([P, nk * HC], F32, tag="wrf",
                                name="wrf", bufs=1)
                nc.sync.dma_start(
                    wr_f[:, 0:nfull * HC].rearrange("p (k c) -> p k c", c=HC),
                    w["wr"][0:nfull * P, :].rearrange("(k p) c -> p k c",
                                                      p=P))
                if KIN % P:
                    nc.sync.dma_start(wr_f[:KIN % P, nfull * HC:nk * HC],
                                      w["wr"][nfull * P:KIN, :])
                wr_b = ap_.tile([P, nk * HC], BF16, tag="wrb",
                                name="wrb", bufs=1)
                nc.vector.tensor_copy(wr_b[:, :], wr_f[:, :])
                wr_b3 = wr_b[:, :].rearrange("p (k c) -> p k c", c=HC)
                nck = len(ckt)
                atB_f = ap_.tile([P, nck * 2 * H], F32, tag="attBf",
                                 name="attBf", bufs=1)
                at3f = atB_f[:, :].rearrange("p (k c) -> p k c", c=2 * H)
                nc.sync.dma_start(
                    at3f[:, :, 0:H],
                    w["attBp"][:, :].rearrange("(k p) c -> p k c", p=P))
                nc.sync.dma_start(
                    at3f[:, :, H:2 * H],
                    w["attB"][:, :].rearrange("(k p) c -> p k c", p=P))
                atB_all = ap_.tile([P, nck * 2 * H], BF16, tag="attB",
                                   name="attB", bufs=1)
                nc.vector.tensor_copy(atB_all[:, :], atB_f[:, :])
                at3 = atB_all[:, :].rearrange("p (k c) -> p k c", c=2 * H)
                for ci, (co, ck) in enumerate(ckt):
                    sp_ = ci == len(ckt) - 1
                    attB_b = at3[:, ci, :]
                    for wsel in (0, 1):  # 0 = wl (permuted attB), 1 = wr
                        wT = ap_.tile([P, KIN], BF16, tag=f"wT{wsel}", name=f"wT{wsel}")
                        for fi, (fo, fk) in enumerate(kk):
                            if wsel == 0:
                                src_ap = wl_sb[fi][:fk, co:co + ck]
                            else:
                                src_ap = wr_b3[:fk, fi, co:co + ck]
                            pt = pp.tile([P, P], BF16, tag="ttb", name="ttb", space="PSUM", bufs=2)
                            nc.tensor.transpose(pt[:ck, :fk], src_ap,
                                                ident_bf[:fk, :fk])
                            nc.vector.tensor_copy(wT[:ck, fo:fo + fk],
                                                  pt[:ck, :fk])
                        attB_t = attB_b[:, 4 * wsel:4 * wsel + 4]
                        for fi, (fo, fk) in enumerate(kk):
                            cs = fi * 8 + 4 * wsel
                            st = ci == 0 and fi == 0 and wsel == 0
                            nc.tensor.matmul(pw[:fk, cs:cs + 4],
                                             lhsT=wT[:ck, fo:fo + fk],
                                             rhs=attB_t[:ck, :], start=st,
                                             stop=sp_, skip_group_check=True)
                        bc_ = blc if wsel == 0 else brc
                        nc.tensor.matmul(pw[:1, nk * 8 + 4 * wsel:
                                                nk * 8 + 4 * wsel + 4],
                                         lhsT=bc_[:ck, ci:ci + 1],
                                         rhs=attB_b[:ck, 4 * wsel:4 * wsel + 4],
                                         start=False,
                                         stop=sp_, skip_group_check=True)
                alar = []
                for fi, (fo, fk) in enumerate(kk):
                    t = wp.tile([P, 8], BF16, tag=f"alar{L}_{fo}",
                                name=f"alar{L}_{fo}")
                    nc.vector.tensor_copy(t[:fk, :], pw[:fk, fi * 8:fi * 8 + 8])
                    alar.append(t)
                alar_b = wp.tile([1, 8], BF16, tag=f"alar_b{L}", name=f"alar_b{L}")
                nc.vector.tensor_copy(alar_b[:, :], pw[:1, nk * 8:nk * 8 + 8])
            return alar, alar_b

        # =========================================================
        def emit_prep_weights(L, wp, pp, cp):
            """Load + preprocess all layer-L weights (no featT deps)."""
            w = wts[L]
            KIN = FIN if L == 1 else HC
            kk = ktiles(KIN)
            nk_ = len(kk)
            nfull_ = KIN // P
            wl_f = cp.tile([P, nk_ * HC], F32, tag="wlf", name="wlf", bufs=1)
            nc.sync.dma_start(
                wl_f[:, 0:nfull_ * HC].rearrange("p (k c) -> p k c", c=HC),
                w["wl"][0:nfull_ * P, :].rearrange("(k p) c -> p k c", p=P))
            if KIN % P:
                nc.sync.dma_start(wl_f[:KIN % P, nfull_ * HC:nk_ * HC],
                                  w["wl"][nfull_ * P:KIN, :])
            wl_sb = []
            for fi, (fo, fk) in enumerate(kk):
                t = wp.tile([P, HC], BF16, tag=f"wl{L}_{fo}", name=f"wl{L}_{fo}")
                nc.vector.tensor_copy(t[:fk, :],
                                      wl_f[:fk, fi * HC:(fi + 1) * HC])
                wl_sb.append(t)
            blrowf = cp.tile([1, HC], F32, tag="blrowf", name="blrowf")
            nc.sync.dma_start(blrowf[:, :], w["bl"][:].rearrange("(o c) -> o c", o=1))
            blrow = wp.tile([1, HC], BF16, tag=f"blrow{L}", name=f"blrow{L}")
            nc.vector.tensor_copy(blrow[:, :], blrowf[:, :])
            alar, alar_b = emit_aux_weights(L, w, KIN, wl_sb, wp, pp)
            att2T_sb = wp.tile([TOPO, H], F32, tag=f"att2T{L}", name=f"att2T{L}")
            nc.sync.dma_start(att2T_sb[:, :], w["att2T"][:, :])
            return dict(kk=kk, wl_sb=wl_sb, blrow=blrow, alar=alar,
                        alar_b=alar_b, att2T_sb=att2T_sb)

        # =========================================================
        def emit_prep(L, featT, wp, pp, cp, ws):
            """featT: list of (ap, k) bf16 feat-major k-tiles [k, NPC]. Emits
            ext_slice + datt_slice for layer L, then the AllGather."""
            kk, wl_sb, blrow = ws["kk"], ws["wl_sb"], ws["blrow"]
            alar, alar_b, att2T_sb = ws["alar"], ws["alar_b"], ws["att2T_sb"]

            tT = topoT[L - 1]
            for bi, (bo, bs) in enumerate(blocks):
                pm = pp.tile([P, HC], F32, tag="acc", name="main", space="PSUM")
                pa = pp.tile([P, 2 * H], F32, tag="aux", name="aux", space="PSUM", bufs=1)
                pta = pp.tile([P, H], F32, tag="ta", name="ta", space="PSUM", bufs=1)
                for i in range(len(kk)):
                    ft, k = featT[i]
                    nc.tensor.matmul(pm[:bs, :], lhsT=ft[:k, bo:bo + bs],
                                     rhs=wl_sb[i][:k, :], start=i == 0,
                                     stop=False, skip_group_check=True)
                    nc.tensor.matmul(pa[:bs, :], lhsT=ft[:k, bo:bo + bs],
                                     rhs=alar[i][:k, :], start=i == 0,
                                     stop=False, skip_group_check=True)
                nc.tensor.matmul(pm[:bs, :], lhsT=ones_bf[:, bo:bo + bs],
                                 rhs=blrow[:, :], start=False, stop=True,
                                 skip_group_check=True)
                nc.tensor.matmul(pa[:bs, :], lhsT=ones_bf[:, bo:bo + bs],
                                 rhs=alar_b[:, :], start=False, stop=True,
                                 skip_group_check=True)
                nc.tensor.matmul(pta[:bs, :], lhsT=tT[:, bo:bo + bs],
                                 rhs=att2T_sb[:, :], start=True, stop=True)
                ext = cp.tile([P, ROW], BF16, tag="ext", name="ext")
                nc.vector.memset(ext[:, HC + 64:ROW], 0.0)
                if L == 1:
                    nc.vector.tensor_copy(ext[:bs, 0:HC], pm[:bs, :])
                else:
                    nc.scalar.copy(ext[:bs, 0:HC], pm[:bs, :])
                extf = ext[:, HC:HC + 64].bitcast(F32)
                nc.vector.memset(extf[:, 24:32], 0.0)
                ptt = pp.tile([P, P], F32, tag="tt", name="tt", space="PSUM", bufs=1)
                nc.tensor.transpose(ptt[:bs, :TOPO], tT[:, bo:bo + bs],
                                    ident[:TOPO, :TOPO])
                nc.scalar.copy(extf[:bs, cfg.C_TOPO:cfg.C_TOPO + TOPO],
                               ptt[:bs, :TOPO])
                nc.vector.memset(extf[:bs, cfg.C_ONE:cfg.C_ONE + 1], 1.0)
                nc.scalar.copy(extf[:bs, cfg.C_AL:cfg.C_AL + H], pa[:bs, 0:H])
                nc.scalar.copy(extf[:bs, cfg.C_TA:cfg.C_TA + H], pta[:bs, :])
                nc.sync.dma_start(ext_sl[L - 1][bo:bo + bs, :], ext[:bs, :])
                datt = cp.tile([P, cfg.DATT], F32, tag="datt", name="datt")
                nc.vector.memset(datt[:, 2 * H:cfg.DATT], 0.0)
                nc.scalar.copy(datt[:bs, 0:H], pa[:bs, H:2 * H])
                nc.scalar.copy(datt[:bs, H:2 * H], pta[:bs, :])
                nc.sync.dma_start(datt_sl[L - 1][bo:bo + bs, :],
                                  datt[:bs, :])
            nc.gpsimd.collective_compute(
                "AllGather", OP.bypass, replica_groups=groups,
                ins=[ext_sl[L - 1][:, :]], outs=[ext_fl[L - 1][:, :]])

        # =========================================================
        def emit_datt(L, dp):
            """Prefetch per-block dst-payload gathers (no AllGather dep)."""
            dts = {}
            for bi, (bo, bs) in enumerate(blocks):
                Tb = schedule[bi]
                base = int(offs[bi])
                dt = dp.tile([P, TMAX * cfg.DATT], F32, tag="D", name="D")
                for go in range(0, Tb, GT_MAX):
                    gn = min(GT_MAX, Tb - go)
                    c0 = (base + go) * 8
                    nc.gpsimd.dma_gather(
                        dt[:, go * cfg.DATT:(go + gn) * cfg.DATT].rearrange(
                            "p (t e) -> p t e", e=cfg.DATT),
                        datt_sl[L - 1][:, :], dst_sb[:, c0:c0 + 8 * gn],
                        num_idxs=P * gn, num_idxs_reg=P * gn,
                        elem_size=cfg.DATT, queue_num=1)
                dts[bi] = dt
            return dts

        # =========================================================
        def emit_edge(L, gp, sp, pp, dts, pool_acc=None):
            """Edge phase for layer L: fills hfm+topoT[1] (L=1) or
            accumulates pool_acc [1, HC] PSUM (L=2)."""
            for bi, (bo, bs) in enumerate(blocks):
                Tb = schedule[bi]
                base = int(offs[bi])
                dt3 = dts[bi][:, :].rearrange("p (t e) -> p t e", e=cfg.DATT)
                # ---- PSUM accumulators for the block ----
                pagg = pp.tile([P, HC], F32, tag="agg", name="agg",
                               space="PSUM", bufs=3)
                psm = pp.tile([P, SMW], F32, tag="small", name="small",
                              space="PSUM")
                for go in range(0, Tb, GT_MAX):
                    gn = min(GT_MAX, Tb - go)
                    g = gp.tile([P, GT_MAX * ROW], BF16, tag="G", name="G")
                    c0 = (base + go) * 8
                    nc.gpsimd.dma_gather(
                        g[:, 0:gn * ROW].rearrange("p (t e) -> p t e", e=ROW),
                        ext_fl[L - 1][:, :], src_sb[:, c0:c0 + 8 * gn],
                        num_idxs=P * gn, num_idxs_reg=P * gn, elem_size=ROW,
                        queue_num=0)
                    g3 = g[:, :].rearrange("p (t r) -> p t r", r=ROW)
                    g3f = g3[:, :, HC:HC + 64].bitcast(F32)
                    # batched logits for the whole gather group
                    lg = sp.tile([P, GT_MAX * 2 * H], F32, tag="lg", name="lg")
                    lg3 = lg[:, :].rearrange("p (t c) -> p t c", c=2 * H)
                    nc.vector.tensor_tensor(
                        lg3[:, 0:gn, :], g3f[:, 0:gn, cfg.C_AL:cfg.C_AL + 2 * H],
                        dt3[:, go:go + gn, 0:2 * H], OP.add)
                    lr = sp.tile([P, GT_MAX * 2 * H], F32, tag="lr", name="lr")
                    nc.vector.tensor_scalar(lr[:, 0:gn * 2 * H],
                                            lg[:, 0:gn * 2 * H], cfg.NEG,
                                            None, OP.mult)
                    nc.vector.tensor_tensor(lr[:, 0:gn * 2 * H],
                                            lr[:, 0:gn * 2 * H],
                                            lg[:, 0:gn * 2 * H], OP.max)
                    etb = sp.tile([P, GT_MAX * 2 * H], BF16, tag="etb",
                                  name="etb")
                    nc.scalar.activation(etb[:, 0:gn * 2 * H],
                                         lr[:, 0:gn * 2 * H], AF.Exp)
                    etb3 = etb[:, :].rearrange("p (t c) -> p t c", c=2 * H)
                    # batched St build for the group
                    stg = sp.tile([P, GT_MAX * P], BF16, tag="S", name="S")
                    stg3 = stg[:, :].rearrange("p (t d) -> p t d", d=P)
                    nc.vector.tensor_tensor(
                        stg3[:, 0:gn, :],
                        iota_bf[:, :].unsqueeze(1).to_broadcast((P, gn, P)),
                        dl_bf[:, base + go:base + go + gn].unsqueeze(2)
                        .to_broadcast((P, gn, P)),
                        OP.is_equal)
                    # batched weighted message matrix R
                    r = sp.tile([P, GT_MAX * (HC + SMW)], BF16, tag="R",
                                name="R")
                    r3 = r[:, :].rearrange("p (t c) -> p t c", c=HC + SMW)
                    nc.vector.tensor_tensor(
                        r3[:, 0:gn, 0:HC].rearrange("p t (c h) -> p t c h", h=H),
                        g3[:, 0:gn, 0:HC].rearrange("p t (c h) -> p t c h", h=H),
                        etb3[:, 0:gn, H:2 * H].unsqueeze(2)
                        .to_broadcast((P, gn, HID, H)),
                        OP.mult)
                    if L == 1:
                        nc.vector.tensor_tensor(
                            r3[:, 0:gn, HC:HC + 64].rearrange(
                                "p t (h c) -> p t h c", c=16),
                            g3f[:, 0:gn, 0:16].unsqueeze(2)
                            .to_broadcast((P, gn, H, 16)),
                            etb3[:, 0:gn, 0:H].unsqueeze(3)
                            .to_broadcast((P, gn, H, 16)),
                            OP.mult)
                        nc.vector.tensor_copy(r3[:, 0:gn, HC + 64:HC + SMW],
                                              etb3[:, 0:gn, H:2 * H])
                    else:
                        nc.vector.tensor_copy(r3[:, 0:gn, HC:HC + H],
                                              etb3[:, 0:gn, H:2 * H])
                    # per-tile St-stationary aggregation matmuls
                    for lt in range(gn):
                        t = go + lt
                        st0, sp1 = t == 0, t == Tb - 1
                        St = stg3[:, lt, :]
                        nc.tensor.matmul(pagg[:, :], lhsT=St,
                                         rhs=r3[:, lt, 0:HC], start=st0,
                                         stop=sp1, skip_group_check=True)
                        nc.tensor.matmul(psm[:, 0:SMW if L == 1 else H],
                                         lhsT=St,
                                         rhs=r3[:, lt, HC:HC + (SMW if L == 1 else H)],
                                         start=st0, stop=sp1,
                                         skip_group_check=True)
                # ---- drain block (all node-major: per-partition math) ----
                nsm = SMW if L == 1 else H
                tsafe = sp.tile([P, SMW], F32, tag="tsafe", name="tsafe")
                nc.vector.tensor_scalar(tsafe[:, 0:nsm], psm[:, 0:nsm], 1e-30,
                                        None, OP.max)
                recS = sp.tile([P, SMW], F32, tag="recS", name="recS")
                nc.vector.reciprocal(recS[:, 0:nsm], tsafe[:, 0:nsm])
                if L == 1:
                    rec2 = recS[:, 64:64 + H]
                else:
                    rec2 = recS[:, 0:H]
                scaled = sp.tile([P, HC], BF16, tag="scaled", name="scaled")
                pagg3 = pagg[:, :].rearrange("p (c h) -> p c h", h=H)
                sc3w = scaled[:, :].rearrange("p (c h) -> p c h", h=H)
                for h in range(H):
                    nc.scalar.activation(sc3w[:, :, h], pagg3[:, :, h],
                                         AF.Copy, scale=rec2[:, h:h + 1])
                if L == 1:
                    # transpose to feat-major hfm with bias add on copy-out
                    sc3 = scaled[:, :].rearrange("p (c h) -> p c h", h=H)
                    for h in range(H):
                        pt = pp.tile([P, P], BF16, tag="ttb", name="ttb",
                                     space="PSUM")
                        nc.tensor.transpose(pt[:, :], sc3[:, :, h], ident_bf)
                        nc.scalar.copy(hfm[h][:, bo:bo + bs], pt[:, :bs])
                    # topo out: sum_h psm[:, 16h+tau]*rec1_h, then /H + bias2
                    rec1 = recS[:, 0:64].rearrange(
                        "p (h s) -> p h s", s=16)[:, :, TOPO:TOPO + 1]
                    tmp1 = sp.tile([P, H * TOPO], F32, tag="tmp1", name="tmp1")
                    t13 = tmp1[:, :].rearrange("p (h s) -> p h s", s=TOPO)
                    nc.vector.tensor_tensor(
                        t13,
                        psm[:, 0:64].rearrange("p (h s) -> p h s", s=16)[:, :, 0:TOPO],
                        rec1.to_broadcast((P, H, TOPO)), OP.mult)
                    t01 = sp.tile([P, TOPO], F32, tag="t01", name="t01")
                    nc.vector.tensor_tensor(t01, t13[:, 0, :], t13[:, 1, :],
                                            OP.add)
                    t23 = sp.tile([P, TOPO], F32, tag="t23", name="t23")
                    nc.vector.tensor_tensor(t23, t13[:, 2, :], t13[:, 3, :],
                                            OP.add)
                    tsum = sp.tile([P, TOPO], BF16, tag="tsum", name="tsum")
                    nc.vector.tensor_tensor(tsum, t01, t23, OP.add)
                    pt2 = pp.tile([P, P], BF16, tag="ttb", name="ttb",
                                  space="PSUM")
                    nc.tensor.transpose(pt2[:TOPO, :], tsum, ident_bf)
                    nc.vector.tensor_scalar(topoT[1][:, bo:bo + bs],
                                            pt2[:TOPO, :bs], 1.0 / H,
                                            b2col[:, 0:1], OP.mult, OP.add)
                else:
                    nc.tensor.matmul(pool_acc[:, :], lhsT=ones_col_bf[:, :],
                                     rhs=scaled[:, :], start=bi == 0,
                                     stop=bi == NBLK - 1,
                                     skip_group_check=True)

        ctx_outer = ExitStack()
        wpW = ctx_outer.enter_context(tc.tile_pool(name="wpW", bufs=1))
        dpW = ctx_outer.enter_context(tc.tile_pool(name="dpW", bufs=8))
        # ================= phase A: layer-1 prep =================
        with tc.tile_pool(name="wpA", bufs=1) as wp, \
             tc.tile_pool(name="ppA", bufs=2, space="PSUM") as pp, \
             tc.tile_pool(name="cpA", bufs=3) as cp, \
             tc.tile_pool(name="xpA", bufs=1) as xp:
            # ---- phase A: x transposes + topo-extractor MLP ----
            xT = [xp.tile([P, NPC], BF16, tag=f"xT{fo}", name=f"xT{fo}")
                  for (fo, fk) in fkt]
            for bi, (bo, bs) in enumerate(blocks):
                xc = cp.tile([P, FIN], F32, tag="xc", name="xc")
                nc.sync.dma_start(xc[:bs, :], x_sl[bo:bo + bs, :])
                xb = cp.tile([P, FIN], BF16, tag="xb", name="xb")
                nc.vector.tensor_copy(xb[:bs, :], xc[:bs, :])
                for fi, (fo, fk) in enumerate(fkt):
                    pt = pp.tile([P, P], BF16, tag="ttb", name="ttb",
                                 space="PSUM", bufs=2)
                    nc.tensor.transpose(pt[:fk, :bs], xb[:bs, fo:fo + fk],
                                        ident_bf[:bs, :bs])
                    nc.vector.tensor_copy(xT[fi][:fk, bo:bo + bs],
                                          pt[:fk, :bs])
            tw1 = []
            for (fo, fk) in fkt:
                tf = cp.tile([P, HID], F32, tag="tw1f", name="tw1f")
                nc.sync.dma_start(tf[:fk, :], te_w1[fo:fo + fk, :])
                t = wp.tile([P, HID], BF16, tag=f"tw1{fo}", name=f"tw1{fo}")
                nc.vector.tensor_copy(t[:fk, :], tf[:fk, :])
                tw1.append(t)
            tb1f = cp.tile([1, HID], F32, tag="tb1f", name="tb1f")
            nc.sync.dma_start(tb1f[:, :], te_b1[:].rearrange("(o c) -> o c", o=1))
            tb1r = wp.tile([1, HID], BF16, tag="tb1r", name="tb1r")
            nc.vector.tensor_copy(tb1r[:, :], tb1f[:, :])
            tw2f = cp.tile([HID, TOPO], F32, tag="tw2f", name="tw2f")
            nc.sync.dma_start(tw2f[:, :], te_w2[:, :])
            tw2 = wp.tile([HID, TOPO], BF16, tag="tw2", name="tw2")
            nc.vector.tensor_copy(tw2[:, :], tw2f[:, :])
            tb2f = cp.tile([1, TOPO], F32, tag="tb2f", name="tb2f")
            nc.sync.dma_start(tb2f[:, :], te_b2[:].rearrange("(o c) -> o c", o=1))
            tb2r = wp.tile([1, TOPO], BF16, tag="tb2r", name="tb2r")
            nc.vector.tensor_copy(tb2r[:, :], tb2f[:, :])
            t_hid = xp.tile([P, NPC], BF16, tag="t_hid", name="t_hid")
            NG = 512
            for go in range(0, NPC, NG):
                gs = min(NG, NPC - go)
                ph = pp.tile([P, NG], F32, tag="acc", name="acc", space="PSUM")
                for fi, (fo, fk) in enumerate(fkt):
                    nc.tensor.matmul(ph[:, :gs], lhsT=tw1[fi][:fk, :],
                                     rhs=xT[fi][:fk, go:go + gs],
                                     start=fi == 0, stop=False,
                                     skip_group_check=True)
                nc.tensor.matmul(ph[:, :gs], lhsT=tb1r[:, :],
                                 rhs=ones_bf[:, go:go + gs], start=False,
                                 stop=True, skip_group_check=True)
                nc.scalar.activation(t_hid[:, go:go + gs], ph[:, :gs], AF.Relu)
                pt = pp.tile([P, NG], F32, tag="acc", name="acc", space="PSUM")
                nc.tensor.matmul(pt[:TOPO, :gs], lhsT=tw2[:, :],
                                 rhs=t_hid[:, go:go + gs], start=True,
                                 stop=False, skip_group_check=True)
                nc.tensor.matmul(pt[:TOPO, :gs], lhsT=tb2r[:, :],
                                 rhs=ones_bf[:, go:go + gs], start=False,
                                 stop=True, skip_group_check=True)
                nc.vector.tensor_copy(topoT[0][:, go:go + gs], pt[:TOPO, :gs])
            featT1 = [(xT[i], fkt[i][1]) for i in range(len(fkt))]
            ws1 = emit_prep_weights(1, wpW, pp, cp)
            ws2 = emit_prep_weights(2, wpW, pp, cp)
            emit_prep(1, featT1, wp, pp, cp, ws1)
            dts1 = emit_datt(1, dpW)

        # ================= phase B: layer-1 edges =================
        with tc.tile_pool(name="gpB", bufs=4) as gp, \
             tc.tile_pool(name="spB", bufs=4) as sp, \
             tc.tile_pool(name="ppB", bufs=2, space="PSUM") as pp:
            emit_edge(1, gp, sp, pp, dts1)

        # ================= phase C: layer-2 prep =================
        with tc.tile_pool(name="wpC", bufs=1) as wp, \
             tc.tile_pool(name="ppC", bufs=2, space="PSUM") as pp, \
             tc.tile_pool(name="cpC", bufs=3) as cp:
            featT2 = [(hfm[h], P) for h in range(H)]
            emit_prep(2, featT2, wp, pp, cp, ws2)
            dts2 = emit_datt(2, dpW)

        # ================= phase D: layer-2 edges + pool =================
        with tc.tile_pool(name="gpD", bufs=4) as gp, \
             tc.tile_pool(name="spD", bufs=4) as sp, \
             tc.tile_pool(name="ppD", bufs=2, space="PSUM") as pp, \
             tc.tile_pool(name="paD", bufs=1, space="PSUM") as pa_:
            pool_acc = pa_.tile([1, HC], F32, tag="pool", name="pool",
                                space="PSUM", bufs=1)
            emit_edge(2, gp, sp, pp, dts2, pool_acc=pool_acc)
            pool_sb = sp.tile([1, HC], F32, tag="pool_sb", name="pool_sb")
            nc.vector.tensor_copy(pool_sb[:, :], pool_acc[:, :])
            nc.sync.dma_start(pool_in[:, :], pool_sb[:, :])

        # ================= phase E: AllReduce + MLP heads =================
        with tc.tile_pool(name="wpE", bufs=1) as wp, \
             tc.tile_pool(name="ppE", bufs=2, space="PSUM") as pp:
            nc.gpsimd.collective_compute(
                "AllGather", OP.bypass, replica_groups=groups,
                ins=[pool_in[:, :]], outs=[pool_out[:, :]])
            pool8 = wp.tile([cfg.CORES, HC], F32, tag="pool8", name="pool8")
            nc.sync.dma_start(pool8[:, :], pool_out[:, :])
            ones8 = wp.tile([cfg.CORES, 1], BF16, tag="ones8", name="ones8")
            nc.vector.memset(ones8[:, :], 1.0)
            pool8b = wp.tile([cfg.CORES, HC], BF16, tag="pool8b",
                             name="pool8b")
            nc.vector.tensor_copy(pool8b[:, :], pool8[:, :])
            psum8 = pp.tile([1, HC], F32, tag="psum8", name="psum8",
                            space="PSUM")
            nc.tensor.matmul(psum8[:, :], lhsT=ones8[:, :], rhs=pool8b[:, :],
                             start=True, stop=True, skip_group_check=True)
            psrow = wp.tile([1, HC], F32, tag="psrow", name="psrow")
            nc.vector.tensor_copy(psrow[:, :], psum8[:, :])
            nc.sync.dma_start(pool_sum[:, :], psrow[:, :])
            # read back as [c(part within chunk), h] column-chunk layout
            pcol = wp.tile([P, H], F32, tag="pcol", name="pcol")
            nc.sync.dma_start(pcol[:, :],
                              pool_sum[:, :].rearrange("o (c h) -> (o c) h",
                                                       h=H))
            pmean = wp.tile([P, H], F32, tag="pmean", name="pmean")
            for h in range(H):
                nc.vector.tensor_scalar(pmean[:, h:h + 1], pcol[:, h:h + 1],
                                        1.0 / N, bcol[2][:, h:h + 1], OP.mult,
                                        OP.add)
            ones1 = wp.tile([1, 1], F32, tag="ones1", name="ones1")
            nc.vector.memset(ones1[:, :], 1.0)
            for nm, out_t in (("v", val_o), ("a", aro_o)):
                hd = heads[nm]
                w1_sb = []
                for ki in range(H):
                    t = wp.tile([P, HID], F32, tag=f"{nm}w1{ki}", name=f"{nm}w1{ki}")
                    nc.sync.dma_start(t[:, :], hd["w1"][ki * P:(ki + 1) * P, :])
                    w1_sb.append(t)
                b1r = wp.tile([1, HID], F32, tag=f"{nm}b1r", name=f"{nm}b1r")
                nc.sync.dma_start(b1r[:, :], hd["b1"][:].rearrange("(o c) -> o c", o=1))
                w2c = wp.tile([HID, 1], F32, tag=f"{nm}w2c", name=f"{nm}w2c")
                nc.sync.dma_start(w2c[:, :], hd["w2"][:, :])
                b2c = wp.tile([1, 1], F32, tag=f"{nm}b2c", name=f"{nm}b2c")
                nc.sync.dma_start(b2c[:, :], hd["b2"][:].rearrange("(o c) -> o c", o=1))
                pm = pp.tile([P, 1], F32, tag="mlp", name="mlp", space="PSUM")
                for ki in range(H):
                    nc.tensor.matmul(pm[:, :], lhsT=w1_sb[ki],
                                     rhs=pmean[:, ki:ki + 1], start=ki == 0,
                                     stop=False, skip_group_check=True)
                nc.tensor.matmul(pm[:, :], lhsT=b1r[:, :],
                                 rhs=ones1[:, :], start=False, stop=True,
                                 skip_group_check=True)
                hv = wp.tile([P, 1], F32, tag=f"{nm}hv", name=f"{nm}hv")
                nc.scalar.activation(hv[:, :], pm[:, :], AF.Relu)
                po = pp.tile([1, 1], F32, tag="mlpo", name="mlpo", space="PSUM")
                nc.tensor.matmul(po[:, :], lhsT=hv[:, :], rhs=w2c[:, :],
                                 start=True, stop=False, skip_group_check=True)
                nc.tensor.matmul(po[:, :], lhsT=b2c[:, :],
                                 rhs=ones1[:, :], start=False, stop=True,
                                 skip_group_check=True)
                ov = wp.tile([1, 1], F32, tag=f"{nm}ov", name=f"{nm}ov")
                nc.vector.tensor_copy(ov[:, :], po[:, :])
                nc.sync.dma_start(out_t[:, :], ov[:, :])

        ctx_outer.close()
    ctx_noncontig.__exit__(None, None, None)
    nc.compile()
    es.close()
    return pr


# --------------------------------------------------------------------------
# entry point
# --------------------------------------------------------------------------

_CACHE = {}


def make_in_maps(inputs, cfg, src_w, dst_w, dl_m):
    x = np.ascontiguousarray(np.asarray(inputs["x"], dtype=np.float32))
    shared = {}
    for k in ("te_w1", "te_b1", "te_w2", "te_b2"):
        shared[k] = np.ascontiguousarray(np.asarray(inputs[k], np.float32))
    for L in (1, 2):
        for k in ("wl", "bl", "wr", "br", "bias", "bias2"):
            shared[f"l{L}_{k}"] = np.ascontiguousarray(
                np.asarray(inputs[f"l{L}_{k}"], np.float32))
        if L == 2:
            # hfm stores layer-1 output WITHOUT its bias; fold bias1 @ W into
            # the layer-2 projection biases instead.
            b1v = np.asarray(inputs["l1_bias"], np.float32)
            shared["l2_bl"] = shared["l2_bl"] + b1v @ shared["l2_wl"]
            shared["l2_br"] = shared["l2_br"] + b1v @ shared["l2_wr"]
        # interleave the feature output layout head-last: col c*H+h <- h*HID+c
        KIN = cfg.FIN if L == 1 else cfg.HC
        shared[f"l{L}_wl"] = np.ascontiguousarray(
            shared[f"l{L}_wl"].reshape(KIN, cfg.H, cfg.HID)
            .transpose(0, 2, 1).reshape(KIN, cfg.HC))
        shared[f"l{L}_bl"] = np.ascontiguousarray(
            shared[f"l{L}_bl"].reshape(cfg.H, cfg.HID).T.reshape(cfg.HC))
        attB = host_attB(inputs[f"l{L}_att"], cfg)
        shared[f"l{L}_attB"] = attB
        shared[f"l{L}_attBp"] = np.ascontiguousarray(
            attB.reshape(cfg.H, cfg.HID, cfg.H).transpose(1, 0, 2)
            .reshape(cfg.HC, cfg.H))
        shared[f"l{L}_att2T"] = np.ascontiguousarray(
            np.asarray(inputs[f"l{L}_att2"], np.float32)
            .reshape(cfg.H, cfg.TOPO).T)
    for nm in ("v", "a"):
        for k in ("w1", "b1", "w2", "b2"):
            shared[f"{nm}_{k}"] = np.ascontiguousarray(
                np.asarray(inputs[f"{nm}_{k}"], np.float32))
    in_maps = []
    for c in range(cfg.CORES):
        m = dict(shared)
        m["x_slice"] = x[c * cfg.NPC:(c + 1) * cfg.NPC].copy()
        m["src_idx"] = np.ascontiguousarray(src_w[c])
        m["dst_idx"] = np.ascontiguousarray(dst_w[c])
        m["dstloc"] = np.ascontiguousarray(dl_m[c])
        in_maps.append(m)
    return in_maps


def run(inputs, cfg=CFG, trace=False):
    schedule, src_w, dst_w, dl_m = host_prep(inputs["edge_index"], cfg)
    key = (cfg.N, cfg.E, tuple(schedule))
    if key not in _CACHE:
        _CACHE[key] = build_program(cfg, schedule)
    pr = _CACHE[key]
    in_maps = make_in_maps(inputs, cfg, src_w, dst_w, dl_m)
    res = run_bass_kernel_spmd(pr.nc, in_maps, list(range(cfg.CORES)),
                               trace=trace)
    out = res.results[0]
    return (np.asarray(out["valence"], np.float32),
            np.asarray(out["arousal"], np.float32)), res


def kernel(**inputs):
    (val, aro), _ = run(inputs)
    return (val, aro)


# revision 59
# speedup vs baseline: 1.0001x; 1.0001x over previous
"""GCATopo (2-layer GTAT GNN) Trainium2 kernel, 8-way SPMD.

Strategy:
 - Nodes partitioned into 8 contiguous ranges (one per core). Edges are
   assigned to the core that owns their dst node, sorted by dst, padded so
   every 128-dst-node block starts at a fresh 128-edge tile. Per-block tile
   counts are shared across cores (SPMD: one program, per-core data).
 - Per layer, each core computes for its node slice a packed "ext" row
   [xl(512) | topo(15) | 1.0 | al(4) | ta(4) | pad] = 576 f32-equiv (1280B
   bf16 row with an f32 payload block) via dense bf16 matmuls (attention
   logit weights folded into the same matmuls), then an AllGather
   replicates the ext table to every core (the halo exchange).
 - Edge phase (St-stationary form): dma_gather pulls src-node ext rows per
   128-edge tile; logits/exp are computed batched across a whole gather
   group (up to 8 tiles per DVE/Act instruction); the weighted message
   matrix R = [feat*e2 | topo*e1 | e2] is built with broadcast multiplies;
   the segment-sum runs on the tensor engine with the one-hot dst-selection
   matrix St as the STATIONARY operand (2 matmuls per tile), producing
   node-major [dst, chan] partials in PSUM; softmax denominators accumulate
   in the same matmuls; normalization becomes per-partition scalar math.
 - Final: mean-pool partials + AllReduce + tiny MLP heads on every core.
"""

from contextlib import ExitStack

import numpy as np

import concourse.bass as bass
import concourse.bacc as bacc
import concourse.tile as tile
from concourse import mybir
from concourse.masks import make_identity
from concourse.bass_utils import run_bass_kernel_spmd

F32 = mybir.dt.float32
BF16 = mybir.dt.bfloat16
I16 = mybir.dt.int16
AF = mybir.ActivationFunctionType
OP = mybir.AluOpType

P = 128


class Cfg:
    def __init__(self, N=20000, E=240000, FIN=576, HID=128, TOPO=15, H=4,
                 CORES=8, NEG=0.2):
        self.N, self.E, self.FIN, self.TOPO, self.H = N, E, FIN, TOPO, H
        self.HID = HID
        self.CORES, self.NEG = CORES, NEG
        self.HC = H * HID                      # 512
        self.ROW = self.HC + 128               # packed ext row (bf16), 1280B
        assert (self.ROW * 2) % 256 == 0
        self.NPC = N // CORES                  # nodes per core
        self.NBLK = (self.NPC + P - 1) // P    # dst blocks per core
        # f32 payload lives in bf16 slots [HC : HC+64) -> f32 view [0:32)
        self.C_TOPO = 0                        # in the f32 view
        self.C_ONE = TOPO
        self.C_AL = 16
        self.C_TA = 20
        self.DATT = 64                         # dst_att row (f32), 256B
        self.SMW = 4 * 16 + H                  # 68: [topo*e1|s1]x4 + s2x4


CFG = Cfg()
GT_MAX = 8  # max tiles (=128 idxs each) per gather call; HW rejects >~1k idxs


def cdiv(a, b):
    return (a + b - 1) // b


def ktiles(F):
    return [(o, min(P, F - o)) for o in range(0, F, P)]


# --------------------------------------------------------------------------
# host-side graph preprocessing (pure indexing on edge_index)
# --------------------------------------------------------------------------

def host_prep(edge_index, cfg):
    N, CORES, NPC, NBLK = cfg.N, cfg.CORES, cfg.NPC, cfg.NBLK
    src = np.asarray(edge_index[0], dtype=np.int64)
    dst = np.asarray(edge_index[1], dtype=np.int64)
    loops = np.arange(N, dtype=np.int64)
    src = np.concatenate([src, loops])
    dst = np.concatenate([dst, loops])
    order = np.argsort(dst, kind="stable")
    s, d = src[order], dst[order]

    core_of = d // NPC
    blk_of = (d % NPC) // P
    counts = np.zeros((CORES, NBLK), dtype=np.int64)
    for c in range(CORES):
        m = core_of == c
        bb = blk_of[m]
        for b in range(NBLK):
            counts[c, b] = int((bb == b).sum())
    schedule = [max(1, cdiv(int(counts[:, b].max()), P)) for b in range(NBLK)]
    offs = np.concatenate([[0], np.cumsum(schedule)]).astype(np.int64)
    ttot = int(offs[-1])

    srcidx = np.zeros((CORES, ttot * P), dtype=np.int16)
    dstidx = np.zeros((CORES, ttot * P), dtype=np.int16)
    dstloc = np.full((CORES, ttot * P), -1.0, dtype=np.float32)
    for c in range(CORES):
        m = core_of == c
        sc, dc, bc = s[m], d[m], blk_of[m]
        for b in range(NBLK):
            mb = bc == b
            n = int(mb.sum())
            base = int(offs[b]) * P
            srcidx[c, base:base + n] = sc[mb].astype(np.int16)
            dstidx[c, base:base + n] = (dc[mb] - c * NPC).astype(np.int16)
            dstloc[c, base:base + n] = (dc[mb] - (c * NPC + b * P)).astype(np.float32)

    # wrap for dma_gather: index i lives at [i % 16, i // 16]; the 16-row
    # block is replicated 8x along partitions (one stripe per gpsimd core)
    src_w = [np.tile(srcidx[c].reshape(-1, 16).T, (8, 1)).copy()
             for c in range(CORES)]
    dst_w = [np.tile(dstidx[c].reshape(-1, 16).T, (8, 1)).copy()
             for c in range(CORES)]
    # dstloc: edge j -> tile j//128, partition j%128
    dl_m = [dstloc[c].reshape(ttot, P).T.copy() for c in range(CORES)]
    return schedule, src_w, dst_w, dl_m


def host_attB(att, cfg):
    """att [1,H,C] -> block-diag [H*C, H] (pure placement of input values)."""
    H, C = cfg.H, cfg.HID
    out = np.zeros((H * C, H), dtype=np.float32)
    a = np.asarray(att, dtype=np.float32).reshape(H, C)
    for h in range(H):
        out[h * C:(h + 1) * C, h] = a[h]
    return out


# --------------------------------------------------------------------------
# program builder
# --------------------------------------------------------------------------

class Prog:
    pass


def build_program(cfg, schedule):
    es = ExitStack()
    nc = bacc.Bacc("TRN2", target_bir_lowering=False, debug=False,
                   num_devices=cfg.CORES, num_swdge_queues=2)
    pr = Prog()
    pr.nc = nc
    N, FIN, HID, TOPO, H, HC, ROW, NPC, NBLK = (
        cfg.N, cfg.FIN, cfg.HID, cfg.TOPO, cfg.H, cfg.HC, cfg.ROW, cfg.NPC,
        cfg.NBLK)
    SMW = cfg.SMW
    TTOT = sum(schedule)
    TMAX = max(schedule)
    W16 = TTOT * P // 16
    groups = [list(range(cfg.CORES))]

    def din(name, shape, dtype=F32):
        return nc.dram_tensor(name, list(shape), dtype, kind="ExternalInput")

    # ---- external inputs ----
    x_sl = din("x_slice", (NPC, FIN))
    te_w1 = din("te_w1", (FIN, HID)); te_b1 = din("te_b1", (HID,))
    te_w2 = din("te_w2", (HID, TOPO)); te_b2 = din("te_b2", (TOPO,))
    wts = {}
    for L in (1, 2):
        KIN = FIN if L == 1 else HC
        wts[L] = dict(
            wl=din(f"l{L}_wl", (KIN, HC)), bl=din(f"l{L}_bl", (HC,)),
            wr=din(f"l{L}_wr", (KIN, HC)), br=din(f"l{L}_br", (HC,)),
            attB=din(f"l{L}_attB", (HC, H)),
            attBp=din(f"l{L}_attBp", (HC, H)),
            att2T=din(f"l{L}_att2T", (TOPO, H)),
            bias=din(f"l{L}_bias", (HC,)), bias2=din(f"l{L}_bias2", (TOPO,)),
        )
    heads = {}
    for nm in ("v", "a"):
        heads[nm] = dict(w1=din(f"{nm}_w1", (HC, HID)), b1=din(f"{nm}_b1", (HID,)),
                         w2=din(f"{nm}_w2", (HID, 1)), b2=din(f"{nm}_b2", (1,)))
    src_i = din("src_idx", (P, W16), I16)
    dst_i = din("dst_idx", (P, W16), I16)
    dl_i = din("dstloc", (P, TTOT))

    # ---- outputs ----
    val_o = nc.dram_tensor("valence", [1, 1], F32, kind="ExternalOutput")
    aro_o = nc.dram_tensor("arousal", [1, 1], F32, kind="ExternalOutput")

    # ---- internal DRAM ----
    ext_sl = [nc.dram_tensor(f"ext_slice{L}", [NPC, ROW], BF16)
              for L in (1, 2)]
    ext_fl = [nc.dram_tensor(f"ext_full{L}", [N, ROW], BF16,
                             addr_space="Shared") for L in (1, 2)]
    datt_sl = [nc.dram_tensor(f"datt_slice{L}", [NPC, cfg.DATT], F32)
               for L in (1, 2)]
    pool_in = nc.dram_tensor("pool_in", [1, HC], F32)
    pool_out = nc.dram_tensor("pool_out", [cfg.CORES, HC], F32,
                              addr_space="Shared")
    pool_sum = nc.dram_tensor("pool_sum", [1, HC], F32)

    blocks = ktiles(NPC)          # node chunks (== dst blocks) per core
    fkt = ktiles(FIN)
    ckt = ktiles(HC)
    offs = np.concatenate([[0], np.cumsum(schedule)]).astype(int)

    ctx_noncontig = nc.allow_non_contiguous_dma("tiny transposed weight loads")
    ctx_noncontig.__enter__()
    with tile.TileContext(nc) as tc:
        # ================= static SBUF =================
        ident = nc.alloc_sbuf_tensor("ident", [P, P], F32).ap()
        make_identity(nc, ident)
        ident_bf = nc.alloc_sbuf_tensor("ident_bf", [P, P], BF16).ap()
        nc.vector.tensor_copy(ident_bf, ident)
        iota_i = nc.alloc_sbuf_tensor("iota_i", [P, P], mybir.dt.int32).ap()
        nc.gpsimd.iota(iota_i, pattern=[[1, P]], base=0, channel_multiplier=0)
        iota_bf = nc.alloc_sbuf_tensor("iota_bf", [P, P], BF16).ap()
        nc.vector.tensor_copy(iota_bf, iota_i)
        ones_col_bf = nc.alloc_sbuf_tensor("ones_col_bf", [P, 1], BF16).ap()
        nc.gpsimd.memset(ones_col_bf, 1.0)
        ones_bf = nc.alloc_sbuf_tensor("ones_bf", [1, NPC], BF16).ap()
        nc.gpsimd.memset(ones_bf, 1.0)

        src_sb = nc.alloc_sbuf_tensor("src_sb", [P, W16], I16).ap()
        dst_sb = nc.alloc_sbuf_tensor("dst_sb", [P, W16], I16).ap()
        dl_sb = nc.alloc_sbuf_tensor("dl_sb", [P, TTOT], F32).ap()
        nc.sync.dma_start(src_sb, src_i[:, :])
        nc.sync.dma_start(dst_sb, dst_i[:, :])
        nc.sync.dma_start(dl_sb, dl_i[:, :])
        dl_bf = nc.alloc_sbuf_tensor("dl_bf", [P, TTOT], BF16).ap()
        nc.vector.tensor_copy(dl_bf, dl_sb)

        topoT = [nc.alloc_sbuf_tensor(f"topoT{L}", [TOPO, NPC], F32).ap()
                 for L in (1, 2)]
        hfm = [nc.alloc_sbuf_tensor(f"hfm{h}", [P, NPC], BF16).ap()
               for h in range(H)]
        b2col = nc.alloc_sbuf_tensor("b2col", [TOPO, 1], F32).ap()
        nc.sync.dma_start(b2col, wts[1]["bias2"][:].rearrange("(t o) -> t o", o=1))
        bcol = {}
        for L in (1, 2):
            bcol[L] = nc.alloc_sbuf_tensor(f"bcol{L}", [P, H], F32).ap()
            nc.sync.dma_start(bcol[L],
                              wts[L]["bias"][:].rearrange("(h c) -> c h", h=H))

        # =========================================================
        def emit_aux_weights(L, w, KIN, wl_sb, wp, pp):
            """[wl@attB | wr@attB] k-tiles [(fk,8)] bf16 + bias row [1,8].

            wl_sb: already-loaded bf16 wl k-tiles (reused as transpose src).
            Layout: cols 0:4 = al weights, 4:8 = ar weights.
            """
            kk = ktiles(KIN)
            nk = len(kk)
            with tc.tile_pool(name=f"aux{KIN}", bufs=2) as ap_:
                pw = pp.tile([P, nk * 8 + 8], F32, tag="auxw", name="auxw", space="PSUM", bufs=1)
                blc = ap_.tile([P, HC // P], BF16, tag="blc", name="blc")
                blcf = ap_.tile([P, HC // P], F32, tag="blcf", name="blcf")
                nc.sync.dma_start(blcf[:, :],
                                  w["bl"][:].rearrange("(a c) -> c a", c=P))
                nc.vector.tensor_copy(blc[:, :], blcf[:, :])
                brc = ap_.tile([P, HC // P], BF16, tag="brc", name="brc")
                brcf = ap_.tile([P, HC // P], F32, tag="brcf", name="brcf")
                nc.sync.dma_start(brcf[:, :],
                                  w["br"][:].rearrange("(a c) -> c a", c=P))
                nc.vector.tensor_copy(brc[:, :], brcf[:, :])
                nfull = KIN // P
                wr_f = ap_.tile([P, nk * HC], F32, tag="wrf",
                                name="wrf", bufs=1)
                nc.sync.dma_start(
                    wr_f[:, 0:nfull * HC].rearrange("p (k c) -> p k c", c=HC),
                    w["wr"][0:nfull * P, :].rearrange("(k p) c -> p k c",
                                                      p=P))
                if KIN % P:
                    nc.sync.dma_start(wr_f[:KIN % P, nfull * HC:nk * HC],
                                      w["wr"][nfull * P:KIN, :])
                wr_b = ap_.tile([P, nk * HC], BF16, tag="wrb",
                                name="wrb", bufs=1)
                nc.vector.tensor_copy(wr_b[:, :], wr_f[:, :])
                wr_b3 = wr_b[:, :].rearrange("p (k c) -> p k c", c=HC)
                nck = len(ckt)
                atB_f = ap_.tile([P, nck * 2 * H], F32, tag="attBf",
                                 name="attBf", bufs=1)
                at3f = atB_f[:, :].rearrange("p (k c) -> p k c", c=2 * H)
                nc.sync.dma_start(
                    at3f[:, :, 0:H],
                    w["attBp"][:, :].rearrange("(k p) c -> p k c", p=P))
                nc.sync.dma_start(
                    at3f[:, :, H:2 * H],
                    w["attB"][:, :].rearrange("(k p) c -> p k c", p=P))
                atB_all = ap_.tile([P, nck * 2 * H], BF16, tag="attB",
                                   name="attB", bufs=1)
                nc.vector.tensor_copy(atB_all[:, :], atB_f[:, :])
                at3 = atB_all[:, :].rearrange("p (k c) -> p k c", c=2 * H)
                for ci, (co, ck) in enumerate(ckt):
                    sp_ = ci == len(ckt) - 1
                    attB_b = at3[:, ci, :]
                    for wsel in (0, 1):  # 0 = wl (permuted attB), 1 = wr
                        wT = ap_.tile([P, KIN], BF16, tag=f"wT{wsel}", name=f"wT{wsel}")
                        for fi, (fo, fk) in enumerate(kk):
                            if wsel == 0:
                                src_ap = wl_sb[fi][:fk, co:co + ck]
                            else:
                                src_ap = wr_b3[:fk, fi, co:co + ck]
                            pt = pp.tile([P, P], BF16, tag="ttb", name="ttb", space="PSUM", bufs=2)
                            nc.tensor.transpose(pt[:ck, :fk], src_ap,
                                                ident_bf[:fk, :fk])
                            nc.vector.tensor_copy(wT[:ck, fo:fo + fk],
                                                  pt[:ck, :fk])
                        attB_t = attB_b[:, 4 * wsel:4 * wsel + 4]
                        for fi, (fo, fk) in enumerate(kk):
                            cs = fi * 8 + 4 * wsel
                            st = ci == 0 and fi == 0 and wsel == 0
                            nc.tensor.matmul(pw[:fk, cs:cs + 4],
                                             lhsT=wT[:ck, fo:fo + fk],
                                             rhs=attB_t[:ck, :], start=st,
                                             stop=sp_, skip_group_check=True)
                        bc_ = blc if wsel == 0 else brc
                        nc.tensor.matmul(pw[:1, nk * 8 + 4 * wsel:
                                                nk * 8 + 4 * wsel + 4],
                                         lhsT=bc_[:ck, ci:ci + 1],
                                         rhs=attB_b[:ck, 4 * wsel:4 * wsel + 4],
                                         start=False,
                                         stop=sp_, skip_group_check=True)
                alar = []
                for fi, (fo, fk) in enumerate(kk):
                    t = wp.tile([P, 8], BF16, tag=f"alar{L}_{fo}",
                                name=f"alar{L}_{fo}")
                    nc.vector.tensor_copy(t[:fk, :], pw[:fk, fi * 8:fi * 8 + 8])
                    alar.append(t)
                alar_b = wp.tile([1, 8], BF16, tag=f"alar_b{L}", name=f"alar_b{L}")
                nc.vector.tensor_copy(alar_b[:, :], pw[:1, nk * 8:nk * 8 + 8])
            return alar, alar_b

        # =========================================================
        def emit_prep_weights(L, wp, pp, cp):
            """Load + preprocess all layer-L weights (no featT deps)."""
            w = wts[L]
            KIN = FIN if L == 1 else HC
            kk = ktiles(KIN)
            wl_sb = []
            for (fo, fk) in kk:
                tf = cp.tile([P, HC], F32, tag="wlf", name="wlf")
                nc.sync.dma_start(tf[:fk, :], w["wl"][fo:fo + fk, :])
                t = wp.tile([P, HC], BF16, tag=f"wl{L}_{fo}", name=f"wl{L}_{fo}")
                nc.vector.tensor_copy(t[:fk, :], tf[:fk, :])
                wl_sb.append(t)
            blrowf = cp.tile([1, HC], F32, tag="blrowf", name="blrowf")
            nc.sync.dma_start(blrowf[:, :], w["bl"][:].rearrange("(o c) -> o c", o=1))
            blrow = wp.tile([1, HC], BF16, tag=f"blrow{L}", name=f"blrow{L}")
            nc.vector.tensor_copy(blrow[:, :], blrowf[:, :])
            alar, alar_b = emit_aux_weights(L, w, KIN, wl_sb, wp, pp)
            att2T_sb = wp.tile([TOPO, H], F32, tag=f"att2T{L}", name=f"att2T{L}")
            nc.sync.dma_start(att2T_sb[:, :], w["att2T"][:, :])
            return dict(kk=kk, wl_sb=wl_sb, blrow=blrow, alar=alar,
                        alar_b=alar_b, att2T_sb=att2T_sb)

        # =========================================================
        def emit_prep(L, featT, wp, pp, cp, ws):
            """featT: list of (ap, k) bf16 feat-major k-tiles [k, NPC]. Emits
            ext_slice + datt_slice for layer L, then the AllGather."""
            kk, wl_sb, blrow = ws["kk"], ws["wl_sb"], ws["blrow"]
            alar, alar_b, att2T_sb = ws["alar"], ws["alar_b"], ws["att2T_sb"]

            tT = topoT[L - 1]
            for bi, (bo, bs) in enumerate(blocks):
                pm = pp.tile([P, HC], F32, tag="acc", name="main", space="PSUM")
                pa = pp.tile([P, 2 * H], F32, tag="aux", name="aux", space="PSUM", bufs=1)
                pta = pp.tile([P, H], F32, tag="ta", name="ta", space="PSUM", bufs=1)
                for i in range(len(kk)):
                    ft, k = featT[i]
                    nc.tensor.matmul(pm[:bs, :], lhsT=ft[:k, bo:bo + bs],
                                     rhs=wl_sb[i][:k, :], start=i == 0,
                                     stop=False, skip_group_check=True)
                    nc.tensor.matmul(pa[:bs, :], lhsT=ft[:k, bo:bo + bs],
                                     rhs=alar[i][:k, :], start=i == 0,
                                     stop=False, skip_group_check=True)
                nc.tensor.matmul(pm[:bs, :], lhsT=ones_bf[:, bo:bo + bs],
                                 rhs=blrow[:, :], start=False, stop=True,
                                 skip_group_check=True)
                nc.tensor.matmul(pa[:bs, :], lhsT=ones_bf[:, bo:bo + bs],
                                 rhs=alar_b[:, :], start=False, stop=True,
                                 skip_group_check=True)
                nc.tensor.matmul(pta[:bs, :], lhsT=tT[:, bo:bo + bs],
                                 rhs=att2T_sb[:, :], start=True, stop=True)
                ext = cp.tile([P, ROW], BF16, tag="ext", name="ext")
                nc.vector.memset(ext[:, HC + 64:ROW], 0.0)
                if L == 1:
                    nc.vector.tensor_copy(ext[:bs, 0:HC], pm[:bs, :])
                else:
                    nc.scalar.copy(ext[:bs, 0:HC], pm[:bs, :])
                extf = ext[:, HC:HC + 64].bitcast(F32)
                nc.vector.memset(extf[:, 24:32], 0.0)
                ptt = pp.tile([P, P], F32, tag="tt", name="tt", space="PSUM", bufs=1)
                nc.tensor.transpose(ptt[:bs, :TOPO], tT[:, bo:bo + bs],
                                    ident[:TOPO, :TOPO])
                nc.scalar.copy(extf[:bs, cfg.C_TOPO:cfg.C_TOPO + TOPO],
                               ptt[:bs, :TOPO])
                nc.vector.memset(extf[:bs, cfg.C_ONE:cfg.C_ONE + 1], 1.0)
                nc.scalar.copy(extf[:bs, cfg.C_AL:cfg.C_AL + H], pa[:bs, 0:H])
                nc.scalar.copy(extf[:bs, cfg.C_TA:cfg.C_TA + H], pta[:bs, :])
                nc.sync.dma_start(ext_sl[L - 1][bo:bo + bs, :], ext[:bs, :])
                datt = cp.tile([P, cfg.DATT], F32, tag="datt", name="datt")
                nc.vector.memset(datt[:, 2 * H:cfg.DATT], 0.0)
                nc.scalar.copy(datt[:bs, 0:H], pa[:bs, H:2 * H])
                nc.scalar.copy(datt[:bs, H:2 * H], pta[:bs, :])
                nc.sync.dma_start(datt_sl[L - 1][bo:bo + bs, :],
                                  datt[:bs, :])
            nc.gpsimd.collective_compute(
                "AllGather", OP.bypass, replica_groups=groups,
                ins=[ext_sl[L - 1][:, :]], outs=[ext_fl[L - 1][:, :]])

        # =========================================================
        def emit_datt(L, dp):
            """Prefetch per-block dst-payload gathers (no AllGather dep)."""
            dts = {}
            for bi, (bo, bs) in enumerate(blocks):
                Tb = schedule[bi]
                base = int(offs[bi])
                dt = dp.tile([P, TMAX * cfg.DATT], F32, tag="D", name="D")
                for go in range(0, Tb, GT_MAX):
                    gn = min(GT_MAX, Tb - go)
                    c0 = (base + go) * 8
                    nc.gpsimd.dma_gather(
                        dt[:, go * cfg.DATT:(go + gn) * cfg.DATT].rearrange(
                            "p (t e) -> p t e", e=cfg.DATT),
                        datt_sl[L - 1][:, :], dst_sb[:, c0:c0 + 8 * gn],
                        num_idxs=P * gn, num_idxs_reg=P * gn,
                        elem_size=cfg.DATT, queue_num=1)
                dts[bi] = dt
            return dts

        # =========================================================
        def emit_edge(L, gp, sp, pp, dts, pool_acc=None):
            """Edge phase for layer L: fills hfm+topoT[1] (L=1) or
            accumulates pool_acc [1, HC] PSUM (L=2)."""
            for bi, (bo, bs) in enumerate(blocks):
                Tb = schedule[bi]
                base = int(offs[bi])
                dt3 = dts[bi][:, :].rearrange("p (t e) -> p t e", e=cfg.DATT)
                # ---- PSUM accumulators for the block ----
                pagg = pp.tile([P, HC], F32, tag="agg", name="agg",
                               space="PSUM", bufs=3)
                psm = pp.tile([P, SMW], F32, tag="small", name="small",
                              space="PSUM")
                for go in range(0, Tb, GT_MAX):
                    gn = min(GT_MAX, Tb - go)
                    g = gp.tile([P, GT_MAX * ROW], BF16, tag="G", name="G")
                    c0 = (base + go) * 8
                    nc.gpsimd.dma_gather(
                        g[:, 0:gn * ROW].rearrange("p (t e) -> p t e", e=ROW),
                        ext_fl[L - 1][:, :], src_sb[:, c0:c0 + 8 * gn],
                        num_idxs=P * gn, num_idxs_reg=P * gn, elem_size=ROW,
                        queue_num=0)
                    g3 = g[:, :].rearrange("p (t r) -> p t r", r=ROW)
                    g3f = g3[:, :, HC:HC + 64].bitcast(F32)
                    # batched logits for the whole gather group
                    lg = sp.tile([P, GT_MAX * 2 * H], F32, tag="lg", name="lg")
                    lg3 = lg[:, :].rearrange("p (t c) -> p t c", c=2 * H)
                    nc.vector.tensor_tensor(
                        lg3[:, 0:gn, :], g3f[:, 0:gn, cfg.C_AL:cfg.C_AL + 2 * H],
                        dt3[:, go:go + gn, 0:2 * H], OP.add)
                    lr = sp.tile([P, GT_MAX * 2 * H], F32, tag="lr", name="lr")
                    nc.vector.tensor_scalar(lr[:, 0:gn * 2 * H],
                                            lg[:, 0:gn * 2 * H], cfg.NEG,
                                            None, OP.mult)
                    nc.vector.tensor_tensor(lr[:, 0:gn * 2 * H],
                                            lr[:, 0:gn * 2 * H],
                                            lg[:, 0:gn * 2 * H], OP.max)
                    etb = sp.tile([P, GT_MAX * 2 * H], BF16, tag="etb",
                                  name="etb")
                    nc.scalar.activation(etb[:, 0:gn * 2 * H],
                                         lr[:, 0:gn * 2 * H], AF.Exp)
                    etb3 = etb[:, :].rearrange("p (t c) -> p t c", c=2 * H)
                    # batched St build for the group
                    stg = sp.tile([P, GT_MAX * P], BF16, tag="S", name="S")
                    stg3 = stg[:, :].rearrange("p (t d) -> p t d", d=P)
                    nc.vector.tensor_tensor(
                        stg3[:, 0:gn, :],
                        iota_bf[:, :].unsqueeze(1).to_broadcast((P, gn, P)),
                        dl_bf[:, base + go:base + go + gn].unsqueeze(2)
                        .to_broadcast((P, gn, P)),
                        OP.is_equal)
                    # batched weighted message matrix R
                    r = sp.tile([P, GT_MAX * (HC + SMW)], BF16, tag="R",
                                name="R")
                    r3 = r[:, :].rearrange("p (t c) -> p t c", c=HC + SMW)
                    nc.vector.tensor_tensor(
                        r3[:, 0:gn, 0:HC].rearrange("p t (c h) -> p t c h", h=H),
                        g3[:, 0:gn, 0:HC].rearrange("p t (c h) -> p t c h", h=H),
                        etb3[:, 0:gn, H:2 * H].unsqueeze(2)
                        .to_broadcast((P, gn, HID, H)),
                        OP.mult)
                    if L == 1:
                        nc.vector.tensor_tensor(
                            r3[:, 0:gn, HC:HC + 64].rearrange(
                                "p t (h c) -> p t h c", c=16),
                            g3f[:, 0:gn, 0:16].unsqueeze(2)
                            .to_broadcast((P, gn, H, 16)),
                            etb3[:, 0:gn, 0:H].unsqueeze(3)
                            .to_broadcast((P, gn, H, 16)),
                            OP.mult)
                        nc.vector.tensor_copy(r3[:, 0:gn, HC + 64:HC + SMW],
                                              etb3[:, 0:gn, H:2 * H])
                    else:
                        nc.vector.tensor_copy(r3[:, 0:gn, HC:HC + H],
                                              etb3[:, 0:gn, H:2 * H])
                    # per-tile St-stationary aggregation matmuls
                    for lt in range(gn):
                        t = go + lt
                        st0, sp1 = t == 0, t == Tb - 1
                        St = stg3[:, lt, :]
                        nc.tensor.matmul(pagg[:, :], lhsT=St,
                                         rhs=r3[:, lt, 0:HC], start=st0,
                                         stop=sp1, skip_group_check=True)
                        nc.tensor.matmul(psm[:, 0:SMW if L == 1 else H],
                                         lhsT=St,
                                         rhs=r3[:, lt, HC:HC + (SMW if L == 1 else H)],
                                         start=st0, stop=sp1,
                                         skip_group_check=True)
                # ---- drain block (all node-major: per-partition math) ----
                nsm = SMW if L == 1 else H
                tsafe = sp.tile([P, SMW], F32, tag="tsafe", name="tsafe")
                nc.vector.tensor_scalar(tsafe[:, 0:nsm], psm[:, 0:nsm], 1e-30,
                                        None, OP.max)
                recS = sp.tile([P, SMW], F32, tag="recS", name="recS")
                nc.vector.reciprocal(recS[:, 0:nsm], tsafe[:, 0:nsm])
                if L == 1:
                    rec2 = recS[:, 64:64 + H]
                else:
                    rec2 = recS[:, 0:H]
                scaled = sp.tile([P, HC], BF16, tag="scaled", name="scaled")
                pagg3 = pagg[:, :].rearrange("p (c h) -> p c h", h=H)
                sc3w = scaled[:, :].rearrange("p (c h) -> p c h", h=H)
                for h in range(H):
                    nc.scalar.activation(sc3w[:, :, h], pagg3[:, :, h],
                                         AF.Copy, scale=rec2[:, h:h + 1])
                if L == 1:
                    # transpose to feat-major hfm with bias add on copy-out
                    sc3 = scaled[:, :].rearrange("p (c h) -> p c h", h=H)
                    for h in range(H):
                        pt = pp.tile([P, P], BF16, tag="ttb", name="ttb",
                                     space="PSUM")
                        nc.tensor.transpose(pt[:, :], sc3[:, :, h], ident_bf)
                        nc.scalar.copy(hfm[h][:, bo:bo + bs], pt[:, :bs])
                    # topo out: sum_h psm[:, 16h+tau]*rec1_h, then /H + bias2
                    rec1 = recS[:, 0:64].rearrange(
                        "p (h s) -> p h s", s=16)[:, :, TOPO:TOPO + 1]
                    tmp1 = sp.tile([P, H * TOPO], F32, tag="tmp1", name="tmp1")
                    t13 = tmp1[:, :].rearrange("p (h s) -> p h s", s=TOPO)
                    nc.vector.tensor_tensor(
                        t13,
                        psm[:, 0:64].rearrange("p (h s) -> p h s", s=16)[:, :, 0:TOPO],
                        rec1.to_broadcast((P, H, TOPO)), OP.mult)
                    t01 = sp.tile([P, TOPO], F32, tag="t01", name="t01")
                    nc.vector.tensor_tensor(t01, t13[:, 0, :], t13[:, 1, :],
                                            OP.add)
                    t23 = sp.tile([P, TOPO], F32, tag="t23", name="t23")
                    nc.vector.tensor_tensor(t23, t13[:, 2, :], t13[:, 3, :],
                                            OP.add)
                    tsum = sp.tile([P, TOPO], BF16, tag="tsum", name="tsum")
                    nc.vector.tensor_tensor(tsum, t01, t23, OP.add)
                    pt2 = pp.tile([P, P], BF16, tag="ttb", name="ttb",
                                  space="PSUM")
                    nc.tensor.transpose(pt2[:TOPO, :], tsum, ident_bf)
                    nc.vector.tensor_scalar(topoT[1][:, bo:bo + bs],
                                            pt2[:TOPO, :bs], 1.0 / H,
                                            b2col[:, 0:1], OP.mult, OP.add)
                else:
                    nc.tensor.matmul(pool_acc[:, :], lhsT=ones_col_bf[:, :],
                                     rhs=scaled[:, :], start=bi == 0,
                                     stop=bi == NBLK - 1,
                                     skip_group_check=True)

        ctx_outer = ExitStack()
        wpW = ctx_outer.enter_context(tc.tile_pool(name="wpW", bufs=1))
        dpW = ctx_outer.enter_context(tc.tile_pool(name="dpW", bufs=8))
        # ================= phase A: layer-1 prep =================
        with tc.tile_pool(name="wpA", bufs=1) as wp, \
             tc.tile_pool(name="ppA", bufs=2, space="PSUM") as pp, \
             tc.tile_pool(name="cpA", bufs=3) as cp, \
             tc.tile_pool(name="xpA", bufs=1) as xp:
            # ---- phase A: x transposes + topo-extractor MLP ----
            xT = [xp.tile([P, NPC], BF16, tag=f"xT{fo}", name=f"xT{fo}")
                  for (fo, fk) in fkt]
            for bi, (bo, bs) in enumerate(blocks):
                xc = cp.tile([P, FIN], F32, tag="xc", name="xc")
                nc.sync.dma_start(xc[:bs, :], x_sl[bo:bo + bs, :])
                xb = cp.tile([P, FIN], BF16, tag="xb", name="xb")
                nc.vector.tensor_copy(xb[:bs, :], xc[:bs, :])
                for fi, (fo, fk) in enumerate(fkt):
                    pt = pp.tile([P, P], BF16, tag="ttb", name="ttb",
                                 space="PSUM", bufs=2)
                    nc.tensor.transpose(pt[:fk, :bs], xb[:bs, fo:fo + fk],
                                        ident_bf[:bs, :bs])
                    nc.vector.tensor_copy(xT[fi][:fk, bo:bo + bs],
                                          pt[:fk, :bs])
            tw1 = []
            for (fo, fk) in fkt:
                tf = cp.tile([P, HID], F32, tag="tw1f", name="tw1f")
                nc.sync.dma_start(tf[:fk, :], te_w1[fo:fo + fk, :])
                t = wp.tile([P, HID], BF16, tag=f"tw1{fo}", name=f"tw1{fo}")
                nc.vector.tensor_copy(t[:fk, :], tf[:fk, :])
                tw1.append(t)
            tb1f = cp.tile([1, HID], F32, tag="tb1f", name="tb1f")
            nc.sync.dma_start(tb1f[:, :], te_b1[:].rearrange("(o c) -> o c", o=1))
            tb1r = wp.tile([1, HID], BF16, tag="tb1r", name="tb1r")
            nc.vector.tensor_copy(tb1r[:, :], tb1f[:, :])
            tw2f = cp.tile([HID, TOPO], F32, tag="tw2f", name="tw2f")
            nc.sync.dma_start(tw2f[:, :], te_w2[:, :])
            tw2 = wp.tile([HID, TOPO], BF16, tag="tw2", name="tw2")
            nc.vector.tensor_copy(tw2[:, :], tw2f[:, :])
            tb2f = cp.tile([1, TOPO], F32, tag="tb2f", name="tb2f")
            nc.sync.dma_start(tb2f[:, :], te_b2[:].rearrange("(o c) -> o c", o=1))
            tb2r = wp.tile([1, TOPO], BF16, tag="tb2r", name="tb2r")
            nc.vector.tensor_copy(tb2r[:, :], tb2f[:, :])
            t_hid = xp.tile([P, NPC], BF16, tag="t_hid", name="t_hid")
            NG = 512
            for go in range(0, NPC, NG):
                gs = min(NG, NPC - go)
                ph = pp.tile([P, NG], F32, tag="acc", name="acc", space="PSUM")
                for fi, (fo, fk) in enumerate(fkt):
                    nc.tensor.matmul(ph[:, :gs], lhsT=tw1[fi][:fk, :],
                                     rhs=xT[fi][:fk, go:go + gs],
                                     start=fi == 0, stop=False,
                                     skip_group_check=True)
                nc.tensor.matmul(ph[:, :gs], lhsT=tb1r[:, :],
                                 rhs=ones_bf[:, go:go + gs], start=False,
                                 stop=True, skip_group_check=True)
                nc.scalar.activation(t_hid[:, go:go + gs], ph[:, :gs], AF.Relu)
                pt = pp.tile([P, NG], F32, tag="acc", name="acc", space="PSUM")
                nc.tensor.matmul(pt[:TOPO, :gs], lhsT=tw2[:, :],
                                 rhs=t_hid[:, go:go + gs], start=True,
                                 stop=False, skip_group_check=True)
                nc.tensor.matmul(pt[:TOPO, :gs], lhsT=tb2r[:, :],
                                 rhs=ones_bf[:, go:go + gs], start=False,
                                 stop=True, skip_group_check=True)
                nc.vector.tensor_copy(topoT[0][:, go:go + gs], pt[:TOPO, :gs])
            featT1 = [(xT[i], fkt[i][1]) for i in range(len(fkt))]
            ws1 = emit_prep_weights(1, wpW, pp, cp)
            ws2 = emit_prep_weights(2, wpW, pp, cp)
            emit_prep(1, featT1, wp, pp, cp, ws1)
            dts1 = emit_datt(1, dpW)

        # ================= phase B: layer-1 edges =================
        with tc.tile_pool(name="gpB", bufs=4) as gp, \
             tc.tile_pool(name="spB", bufs=4) as sp, \
             tc.tile_pool(name="ppB", bufs=2, space="PSUM") as pp:
            emit_edge(1, gp, sp, pp, dts1)

        # ================= phase C: layer-2 prep =================
        with tc.tile_pool(name="wpC", bufs=1) as wp, \
             tc.tile_pool(name="ppC", bufs=2, space="PSUM") as pp, \
             tc.tile_pool(name="cpC", bufs=3) as cp:
            featT2 = [(hfm[h], P) for h in range(H)]
            emit_prep(2, featT2, wp, pp, cp, ws2)
            dts2 = emit_datt(2, dpW)

        # ================= phase D: layer-2 edges + pool =================
        with tc.tile_pool(name="gpD", bufs=4) as gp, \
             tc.tile_pool(name="spD", bufs=4) as sp, \
             tc.tile_pool(name="ppD", bufs=2, space="PSUM") as pp, \
             tc.tile_pool(name="paD", bufs=1, space="PSUM") as pa_:
            pool_acc = pa_.tile([1, HC], F32, tag="pool", name="pool",
                                space="PSUM", bufs=1)
            emit_edge(2, gp, sp, pp, dts2, pool_acc=pool_acc)
            pool_sb = sp.tile([1, HC], F32, tag="pool_sb", name="pool_sb")
            nc.vector.tensor_copy(pool_sb[:, :], pool_acc[:, :])
            nc.sync.dma_start(pool_in[:, :], pool_sb[:, :])

        # ================= phase E: AllReduce + MLP heads =================
        with tc.tile_pool(name="wpE", bufs=1) as wp, \
             tc.tile_pool(name="ppE", bufs=2, space="PSUM") as pp:
            nc.gpsimd.collective_compute(
                "AllGather", OP.bypass, replica_groups=groups,
                ins=[pool_in[:, :]], outs=[pool_out[:, :]])
            pool8 = wp.tile([cfg.CORES, HC], F32, tag="pool8", name="pool8")
            nc.sync.dma_start(pool8[:, :], pool_out[:, :])
            ones8 = wp.tile([cfg.CORES, 1], BF16, tag="ones8", name="ones8")
            nc.vector.memset(ones8[:, :], 1.0)
            pool8b = wp.tile([cfg.CORES, HC], BF16, tag="pool8b",
                             name="pool8b")
            nc.vector.tensor_copy(pool8b[:, :], pool8[:, :])
            psum8 = pp.tile([1, HC], F32, tag="psum8", name="psum8",
                            space="PSUM")
            nc.tensor.matmul(psum8[:, :], lhsT=ones8[:, :], rhs=pool8b[:, :],
                             start=True, stop=True, skip_group_check=True)
            psrow = wp.tile([1, HC], F32, tag="psrow", name="psrow")
            nc.vector.tensor_copy(psrow[:, :], psum8[:, :])
            nc.sync.dma_start(pool_sum[:, :], psrow[:, :])
            # read back as [c(part within chunk), h] column-chunk layout
            pcol = wp.tile([P, H], F32, tag="pcol", name="pcol")
            nc.sync.dma_start(pcol[:, :],
                              pool_sum[:, :].rearrange("o (c h) -> (o c) h",
                                                       h=H))
            pmean = wp.tile([P, H], F32, tag="pmean", name="pmean")
            for h in range(H):
                nc.vector.tensor_scalar(pmean[:, h:h + 1], pcol[:, h:h + 1],
                                        1.0 / N, bcol[2][:, h:h + 1], OP.mult,
                                        OP.add)
            ones1 = wp.tile([1, 1], F32, tag="ones1", name="ones1")
            nc.vector.memset(ones1[:, :], 1.0)
            for nm, out_t in (("v", val_o), ("a", aro_o)):
                hd = heads[nm]
                w1_sb = []
                for ki in range(H):
                    t = wp.tile([P, HID], F32, tag=f"{nm}w1{ki}", name=f"{nm}w1{ki}")
                    nc.sync.dma_start(t[:, :], hd["w1"][ki * P:(ki + 1) * P, :])
                    w1_sb.append(t)
                b1r = wp.tile([1, HID], F32, tag=f"{nm}b1r", name=f"{nm}b1r")
                nc.sync.dma_start(b1r[:, :], hd["b1"][:].rearrange("(o c) -> o c", o=1))
                w2c = wp.tile([HID, 1], F32, tag=f"{nm}w2c", name=f"{nm}w2c")
                nc.sync.dma_start(w2c[:, :], hd["w2"][:, :])
                b2c = wp.tile([1, 1], F32, tag=f"{nm}b2c", name=f"{nm}b2c")
                nc.sync.dma_start(b2c[:, :], hd["b2"][:].rearrange("(o c) -> o c", o=1))
                pm = pp.tile([P, 1], F32, tag="mlp", name="mlp", space="PSUM")
                for ki in range(H):
                    nc.tensor.matmul(pm[:, :], lhsT=w1_sb[ki],
                                     rhs=pmean[:, ki:ki + 1], start=ki == 0,
                                     stop=False, skip_group_check=True)
                nc.tensor.matmul(pm[:, :], lhsT=b1r[:, :],
                                 rhs=ones1[:, :], start=False, stop=True,
                                 skip_group_check=True)
                hv = wp.tile([P, 1], F32, tag=f"{nm}hv", name=f"{nm}hv")
                nc.scalar.activation(hv[:, :], pm[:, :], AF.Relu)
                po = pp.tile([1, 1], F32, tag="mlpo", name="mlpo", space="PSUM")
                nc.tensor.matmul(po[:, :], lhsT=hv[:, :], rhs=w2c[:, :],
                                 start=True, stop=False, skip_group_check=True)
                nc.tensor.matmul(po[:, :], lhsT=b2c[:, :],
                                 rhs=ones1[:, :], start=False, stop=True,
                                 skip_group_check=True)
                ov = wp.tile([1, 1], F32, tag=f"{nm}ov", name=f"{nm}ov")
                nc.vector.tensor_copy(ov[:, :], po[:, :])
                nc.sync.dma_start(out_t[:, :], ov[:, :])

        ctx_outer.close()
    ctx_noncontig.__exit__(None, None, None)
    nc.compile()
    es.close()
    return pr


# --------------------------------------------------------------------------
# entry point
# --------------------------------------------------------------------------

_CACHE = {}


def make_in_maps(inputs, cfg, src_w, dst_w, dl_m):
    x = np.ascontiguousarray(np.asarray(inputs["x"], dtype=np.float32))
    shared = {}
    for k in ("te_w1", "te_b1", "te_w2", "te_b2"):
        shared[k] = np.ascontiguousarray(np.asarray(inputs[k], np.float32))
    for L in (1, 2):
        for k in ("wl", "bl", "wr", "br", "bias", "bias2"):
            shared[f"l{L}_{k}"] = np.ascontiguousarray(
                np.asarray(inputs[f"l{L}_{k}"], np.float32))
        if L == 2:
            # hfm stores layer-1 output WITHOUT its bias; fold bias1 @ W into
            # the layer-2 projection biases instead.
            b1v = np.asarray(inputs["l1_bias"], np.float32)
            shared["l2_bl"] = shared["l2_bl"] + b1v @ shared["l2_wl"]
            shared["l2_br"] = shared["l2_br"] + b1v @ shared["l2_wr"]
        # interleave the feature output layout head-last: col c*H+h <- h*HID+c
        KIN = cfg.FIN if L == 1 else cfg.HC
        shared[f"l{L}_wl"] = np.ascontiguousarray(
            shared[f"l{L}_wl"].reshape(KIN, cfg.H, cfg.HID)
            .transpose(0, 2, 1).reshape(KIN, cfg.HC))
        shared[f"l{L}_bl"] = np.ascontiguousarray(
            shared[f"l{L}_bl"].reshape(cfg.H, cfg.HID).T.reshape(cfg.HC))
        attB = host_attB(inputs[f"l{L}_att"], cfg)
        shared[f"l{L}_attB"] = attB
        shared[f"l{L}_attBp"] = np.ascontiguousarray(
            attB.reshape(cfg.H, cfg.HID, cfg.H).transpose(1, 0, 2)
            .reshape(cfg.HC, cfg.H))
        shared[f"l{L}_att2T"] = np.ascontiguousarray(
            np.asarray(inputs[f"l{L}_att2"], np.float32)
            .reshape(cfg.H, cfg.TOPO).T)
    for nm in ("v", "a"):
        for k in ("w1", "b1", "w2", "b2"):
            shared[f"{nm}_{k}"] = np.ascontiguousarray(
                np.asarray(inputs[f"{nm}_{k}"], np.float32))
    in_maps = []
    for c in range(cfg.CORES):
        m = dict(shared)
        m["x_slice"] = x[c * cfg.NPC:(c + 1) * cfg.NPC].copy()
        m["src_idx"] = np.ascontiguousarray(src_w[c])
        m["dst_idx"] = np.ascontiguousarray(dst_w[c])
        m["dstloc"] = np.ascontiguousarray(dl_m[c])
        in_maps.append(m)
    return in_maps


def run(inputs, cfg=CFG, trace=False):
    schedule, src_w, dst_w, dl_m = host_prep(inputs["edge_index"], cfg)
    key = (cfg.N, cfg.E, tuple(schedule))
    if key not in _CACHE:
        _CACHE[key] = build_program(cfg, schedule)
    pr = _CACHE[key]
    in_maps = make_in_maps(inputs, cfg, src_w, dst_w, dl_m)
    res = run_bass_kernel_spmd(pr.nc, in_maps, list(range(cfg.CORES)),
                               trace=trace)
    out = res.results[0]
    return (np.asarray(out["valence"], np.float32),
            np.asarray(out["arousal"], np.float32)), res


def kernel(**inputs):
    (val, aro), _ = run(inputs)
    return (val, aro)
